# revision 7
# baseline (speedup 1.0000x reference)
"""Trainium2 kernel for nn_AdaptivePoolOrGaussian.

Reference computes, per (batch, channel) image X (256x256):
    out = sum_i w_i * (K_i conv X),  w = softmax(alpha)
where the 8 K_i are separable symmetric 11-tap 2D kernels
(5 avg-pools incl. identity + 3 Gaussians), zero-padded "same" convs.

Math: all 8 tap vectors are even-symmetric 11-vectors, so the combined
operator M = sum_i w_i g_i g_i^T (11x11) has rank <= 6. An identity
shift s*delta@delta^T is peeled off and applied exactly as "+ s*X"
during output evacuation; s is optimized jointly with the rank-R
eigendecomposition of the remainder (alternating s / eigh), and R is
the smallest rank whose predicted white-noise rel err fits the budget
(R=3 for nominal inputs):
    out = sum_r lam_r * conv_H(q_r) conv_W(q_r) X + s X.
Each 1D conv along a 256-long axis is a banded 256x256 matmul; band
structure lets each 128-row k-tile stream only 134 of 256 output
columns. Sharding is pure data parallel: core i owns batch element i.
Per channel: stage A (conv H) matmuls X^T Q_r into one R*512-col PSUM
tile, evacuated to SBUF fp16; stage B (conv W) accumulates
sum_r Y_r (lam_r/s Q_r) in PSUM over all ranks, then VectorE adds the
prescaled s*X during the PSUM->SBUF copy. Compute dtype fp16, PSUM
accumulates fp32. The channel loop is software-pipelined (stage B one
channel behind stage A) so PSUM evacuations hide behind a full channel
of PE work.

Perf notes (from NTFF traces): the NEFF has a fixed ~10us head (all-
engine barrier behind gpsimd boot + DGE start latency) before data
DMAs move, so a ~6us burst of dummy matmuls on uninitialized SBUF
(their PSUM output is never read) bridges the PE through it and HAM
un-throttles (1.2->2.4 GHz) before the first real matmul. x is host-
transposed to [h, c, w] so input DMA lines are sz*512B contiguous.
Outputs are issued on the sync engine (hardware DGE; gpsimd's software
DGE path measured ~40% slower and left a multi-us drain tail). The
PSUM->SBUF evacuation alternates by channel parity (even: ScalarE
1024 cols + VectorE 512; odd: ScalarE all 1536) so both copy engines
average below the PE's ~1.39us/channel; per-op fixed costs (~250ns
ScalarE, ~150ns VectorE) make finer splits lose.
"""

import numpy as np

import concourse.bass as bass
import concourse.tile as tile
from concourse import mybir
from concourse.bass_utils import run_bass_kernel_spmd

N_CORES = 8
C, H, W = 64, 256, 256
KS, HALF = 11, 5
TRIM = 134              # streamed cols per k-tile (even width, 8B-aligned dst)
TRIM_OFF = (0, 122)     # dst col offset per k-tile; overlap accumulates in PSUM
GC = 8                  # channels per DMA group
REL_TARGET = 1.55e-2    # white-noise rel-err budget for eigen truncation
N_WARM = 34             # warm-up matmuls (256 cols each) bridging the DMA head


def _split_sync_waits(nc: bass.Bass, max_waits: int = 1):
    """walrus in this env encodes at most one sync-wait command per
    instruction; move excess waits onto preceding same-engine NOPs
    (engine queues are in-order, so semantics are preserved)."""
    for f in nc.m.functions:
        for bb in list(f.blocks):
            insts = list(bb.instructions)
            new_insts = []
            changed = False
            for inst in insts:
                si = inst.sync_info
                waits = list(si.on_wait) if si is not None and si.on_wait else []
                if len(waits) > max_waits:
                    extra, keep = waits[:-max_waits], waits[-max_waits:]
                    for w in extra:
                        nop = mybir.InstNoOp(
                            name=nc.get_next_instruction_name(), ins=[], outs=[]
                        )
                        nop.engine = inst.engine
                        nop.sync_info = mybir.SyncInfo(on_wait=[w], on_update=[])
                        nc.register_instruction(nop)
                        new_insts.append(nop)
                    si.on_wait = keep
                    changed = True
                new_insts.append(inst)
            if changed:
                bb.instructions = new_insts


def _host_filters(sigmas: np.ndarray, alpha: np.ndarray):
    """Eigendecompose the combined 2D smoothing operator.

    Returns (qa, qb, R, s): packed banded filter blocks for stage A / B,
    each (128, 2*R*TRIM) float16, plus the identity shift s.
    """
    al = alpha.astype(np.float64)
    wts = np.exp(al - al.max())
    wts /= wts.sum()

    gs = np.zeros((8, KS))
    gs[0, HALF] = 1.0                                   # identity (k=0)
    for i, k in enumerate((1, 2, 3, 5), start=1):       # avg pools
        gs[i, HALF - k : HALF + k + 1] = 1.0 / (2 * k + 1)
    ax = np.arange(KS, dtype=np.float64) - (KS - 1) / 2.0
    for i in range(3):                                  # gaussians
        s = abs(float(sigmas[i])) + 1e-6
        g = np.exp(-0.5 * (ax / s) ** 2)
        gs[5 + i] = g / g.sum()

    # The device graph unconditionally adds s*X (the host-prescaled input)
    # at output evacuation, so the eigen part must represent
    # M' = M - s*delta@delta. s is a free parameter: alternate eigh /
    # s = delta^T(M - rank_R)delta to minimize the rank-R residual, and
    # take the smallest R whose predicted white-noise rel err (residual
    # Frobenius over ||M||_F) fits the budget. Clamp s away from 0 so
    # qb = lam/s stays in fp16 range for degenerate softmax weights
    # (M' then goes indefinite, which the |lam| ordering handles).
    w0 = float(wts[0])
    M = (gs.T * wts) @ gs                               # 11x11, rank<=6
    MF = np.linalg.norm(M)
    delta = gs[0]
    for R in range(1, 7):
        s_id = min(max(w0, 1e-2), 1.0)
        for _ in range(60):
            Mr = M - s_id * np.outer(delta, delta)
            lam, V = np.linalg.eigh(Mr)
            order = np.argsort(-np.abs(lam))
            lam, V = lam[order], V[:, order]
            A = (V[:, :R] * lam[:R]) @ V[:, :R].T
            s_new = min(max(float((M - A)[HALF, HALF]), 1e-2), 1.0)
            if abs(s_new - s_id) < 1e-12:
                break
            s_id = s_new
        if np.sqrt(np.sum(lam[R:] ** 2)) < REL_TARGET * MF or R == 6:
            break

    def band(q):
        Q = np.zeros((H, H))
        for d in range(-HALF, HALF + 1):
            i = np.arange(max(0, -d), min(H, H - d))
            Q[i, i + d] = q[d + HALF]
        return Q

    def pack(mats):
        out = np.zeros((128, 2 * R * TRIM), np.float16)
        for kt in range(2):
            for r, Q in enumerate(mats):
                blk = Q[kt * 128 : (kt + 1) * 128, TRIM_OFF[kt] : TRIM_OFF[kt] + TRIM]
                out[:, (kt * R + r) * TRIM : (kt * R + r + 1) * TRIM] = blk.astype(
                    np.float16
                )
        return out

    qa = pack([band(V[:, r]) for r in range(R)])
    qb = pack([band(V[:, r] * (lam[r] / s_id)) for r in range(R)])
    return qa, qb, R, s_id


def _build_nc(R: int) -> bass.Bass:
    nc = bass.Bass()
    # x / out are host-transposed to [h, c, w] so each DMA moves
    # sz-channel * 512B contiguous lines per partition.
    x = nc.declare_dram_parameter("x", [H, C, W], mybir.dt.float16, isOutput=False)
    qa = nc.declare_dram_parameter(
        "qa", [128, 2 * R * TRIM], mybir.dt.float16, isOutput=False
    )
    qb = nc.declare_dram_parameter(
        "qb", [128, 2 * R * TRIM], mybir.dt.float16, isOutput=False
    )
    out = nc.declare_dram_parameter("out", [H, C, W], mybir.dt.float16, isOutput=True)

    f16, f32 = mybir.dt.float16, mybir.dt.float32
    AW = R * 512             # stage-A PSUM/Y width (R banks as fp32)
    # PSUM budget (8 banks): stage-A tile is R banks; double-buffer when
    # 2*R + 2 (pso) fits, else single-buffer (R in 4..6 fallback).
    psa_bufs = 2 if 2 * R + 2 <= 8 else 1

    with tile.TileContext(nc) as tc:
        with (
            tc.tile_pool(name="consts", bufs=1) as consts,
            tc.tile_pool(name="xin", bufs=3) as xin,
            tc.tile_pool(name="ysb", bufs=3) as ysb,
            tc.tile_pool(name="osb", bufs=2) as osb,
            tc.tile_pool(name="psa", bufs=psa_bufs, space="PSUM") as psa,
            tc.tile_pool(name="pso", bufs=2, space="PSUM") as pso,
        ):
            qa_sb = consts.tile([128, 2 * R * TRIM], f16)
            qb_sb = consts.tile([128, 2 * R * TRIM], f16)
            nc.scalar.dma_start(out=qa_sb[:, :], in_=qa[:, :])
            nc.scalar.dma_start(out=qb_sb[:, :], in_=qb[:, :])

            # PE warm-up: the NEFF start barrier + DGE latency keep data
            # DMAs from landing until ~10us; HAM starts the PE at 1.2 GHz
            # and un-throttles after ~3.4us of sustained activity. Burn
            # dummy matmuls across the DMA head so the real matmuls start
            # at 2.4 GHz (their PSUM output is never read).
            scratch = consts.tile([128, 256], f16, name="scratch")
            nc.gpsimd.memset(scratch[:, :], 0.0)
            wtile = [
                pso.tile([128, 512], f32, name=f"warm{i}", tag="po") for i in range(2)
            ]
            for i in range(N_WARM):
                nc.tensor.matmul(
                    wtile[i % 2][:, 0:256],
                    lhsT=scratch[:, 0:128],
                    rhs=scratch[:, 0:256],
                    start=True,
                    stop=True,
                )
            # input groups: small first so PE starts early; output groups:
            # small last so the final store DMA is short
            in_sizes = [1, 2, 2, 3] + [GC] * ((C - 8) // GC)
            out_sizes = [GC] * ((C - 8) // GC) + [4, 2, 1, 1]
            def group_map(sizes):
                m, start = {}, 0
                for gi, sz in enumerate(sizes):
                    for off in range(sz):
                        m[start + off] = (gi, off, start, sz)
                    start += sz
                return m
            in_map, out_map = group_map(in_sizes), group_map(out_sizes)

            xgs: dict[int, object] = {}
            ogs: dict[int, object] = {}
            ys_by_c: dict[int, object] = {}

            def stage_a(c):
                g, ci, c0, sz = in_map[c]
                if ci == 0:
                    # xg[p, (t, c, w)] = x[t*128+p, c, w]; one DMA per
                    # k-tile t keeps (c, w) lines contiguous on both sides.
                    xg = xin.tile([128, sz * 512], f16, name=f"xg{g}", tag="xg")
                    for t in range(2):
                        nc.sync.dma_start(
                            out=xg[:, t * sz * 256 : (t + 1) * sz * 256].rearrange(
                                "p (c w) -> p c w", c=sz
                            ),
                            in_=x[t * 128 : (t + 1) * 128, c0 : c0 + sz, :],
                        )
                    xgs[g] = xg
                xg = xgs[g]
                # stage A: Y_r^T = X^T Q_r (contract H on partitions). All
                # R ranks share one R-bank PSUM tile (rank r at col r*512;
                # no matmul window crosses a bank boundary); (kt, mt)
                # outer so consecutive MMs share the stationary X chunk.
                pa = psa.tile([128, AW], f32, name="pa", tag="pa")
                for kt in range(2):
                    for mt in range(2):
                        base = kt * sz * 256 + ci * 256 + mt * 128
                        lhs = xg[:, base : base + 128]
                        for r in range(R):
                            dst = r * 512 + mt * 256 + TRIM_OFF[kt]
                            nc.tensor.matmul(
                                pa[:, dst : dst + TRIM],
                                lhsT=lhs,
                                rhs=qa_sb[
                                    :, (kt * R + r) * TRIM : (kt * R + r + 1) * TRIM
                                ],
                                start=(kt == 0 and mt == 0),
                                stop=(kt == 1 and mt == 1),
                            )
                # evacuate PSUM -> SBUF f16, alternating by parity: even
                # channels ScalarE [0:1024] + VectorE [1024:AW] (different
                # banks), odd channels ScalarE everything — keeps both
                # engines' per-channel averages under the PE time without
                # paying per-op fixed costs on finer splits.
                ys = ysb.tile([128, AW], f16, name="y", tag="y")
                if c % 2 == 0 and R >= 3:
                    nc.scalar.copy(out=ys[:, :1024], in_=pa[:, :1024])
                    nc.vector.tensor_copy(out=ys[:, 1024:AW], in_=pa[:, 1024:AW])
                else:
                    nc.scalar.copy(out=ys[:, :AW], in_=pa[:, :AW])
                ys_by_c[c] = ys

            def stage_b(c):
                g, ci, c0, sz = out_map[c]
                if ci == 0:
                    ogs[g] = osb.tile([128, sz * 512], f16, name=f"og{g}", tag="og")
                ys = ys_by_c.pop(c)
                # stage B: out = sum_r Y_r (lam_r Q_r)  (contract W)
                po = pso.tile([128, 512], f32, tag="po")
                for r in range(R):
                    for kt in range(2):
                        for mt in range(2):
                            dst = mt * 256 + TRIM_OFF[kt]
                            src = r * 512 + kt * 256 + mt * 128
                            nc.tensor.matmul(
                                po[:, dst : dst + TRIM],
                                lhsT=ys[:, src : src + 128],
                                rhs=qb_sb[
                                    :, (kt * R + r) * TRIM : (kt * R + r + 1) * TRIM
                                ],
                                start=(r == 0 and kt == 0 and mt == 0),
                                stop=(r == R - 1 and kt == 1 and mt == 1),
                            )
                gi, cii, _, szi = in_map[c]
                # og[p, (t, c, w)] = po + s*X, one add per k-tile t (po
                # cols are (t, w); xg/og blocks are t-major).
                for t in range(2):
                    nc.vector.tensor_add(
                        ogs[g][:, t * sz * 256 + ci * 256 : t * sz * 256 + (ci + 1) * 256],
                        po[:, t * 256 : (t + 1) * 256],
                        xgs[gi][:, t * szi * 256 + cii * 256 : t * szi * 256 + (cii + 1) * 256],
                    )
                if ci == sz - 1:
                    og = ogs.pop(g)
                    for t in range(2):
                        nc.sync.dma_start(
                            out=out[t * 128 : (t + 1) * 128, c0 : c0 + sz, :],
                            in_=og[:, t * sz * 256 : (t + 1) * sz * 256].rearrange(
                                "p (c w) -> p c w", c=sz
                            ),
                        )

            # software pipeline: B(c-1) is emitted after A(c), so stage-A
            # evacuations have a full channel of PE work to hide behind
            for c in range(C):
                stage_a(c)
                if c > 0:
                    stage_b(c - 1)
            stage_b(C - 1)
    _split_sync_waits(nc)
    return nc


_NC_CACHE: dict[int, bass.Bass] = {}


def _get_nc(R: int) -> bass.Bass:
    if R not in _NC_CACHE:
        _NC_CACHE[R] = _build_nc(R)
    return _NC_CACHE[R]


def _run(x, sigmas, alpha, trace=False):
    qa, qb, R, s_id = _host_filters(np.asarray(sigmas), np.asarray(alpha))
    # device computes (1/s) * (sum_r Q_r (s X) (lam_r/s) Q_r + s X);
    # scaling X by s up front makes the identity term a plain add at evac
    x = (np.asarray(x, dtype=np.float32) * np.float32(s_id)).astype(np.float16)
    # [b, c, h, w] -> per-core [h, c, w] so DMA lines are contiguous
    xt = np.ascontiguousarray(x.transpose(0, 2, 1, 3))
    nc = _get_nc(R)
    in_maps = [{"x": xt[i], "qa": qa, "qb": qb} for i in range(N_CORES)]
    res = run_bass_kernel_spmd(
        nc, in_maps, core_ids=list(range(N_CORES)), trace=trace
    )
    out = np.stack([res.results[i]["out"] for i in range(N_CORES)])
    out = out.astype(np.float32).transpose(0, 2, 1, 3)
    return np.ascontiguousarray(out), res.exec_time_ns


def kernel(x, sigmas, alpha):
    out, _ = _run(x, sigmas, alpha, trace=False)
    return out


# revision 9
# speedup vs baseline: 1.0022x; 1.0022x over previous
"""Trainium2 kernel for nn_AdaptivePoolOrGaussian.

Reference computes, per (batch, channel) image X (256x256):
    out = sum_i w_i * (K_i conv X),  w = softmax(alpha)
where the 8 K_i are separable symmetric 11-tap 2D kernels
(5 avg-pools incl. identity + 3 Gaussians), zero-padded "same" convs.

Math: all 8 tap vectors are even-symmetric 11-vectors, so the combined
operator M = sum_i w_i g_i g_i^T (11x11) has rank <= 6. An identity
shift s*delta@delta^T is peeled off and applied exactly as "+ s*X"
during output evacuation; s is optimized jointly with the rank-R
eigendecomposition of the remainder (alternating s / eigh), and R is
the smallest rank whose predicted white-noise rel err fits the budget
(R=3 for nominal inputs):
    out = sum_r lam_r * conv_H(q_r) conv_W(q_r) X + s X.
Each 1D conv along a 256-long axis is a banded 256x256 matmul; band
structure lets each 128-row k-tile stream only 134 of 256 output
columns. Sharding is pure data parallel: core i owns batch element i.
Per channel: stage A (conv H) matmuls X^T Q_r into PSUM, evacuated to
SBUF fp16; stage B (conv W) accumulates sum_r Y_r (lam_r/s Q_r) in
PSUM over all ranks, then VectorE adds the prescaled s*X during the
PSUM->SBUF copy. Compute dtype fp16, PSUM accumulates fp32. The
channel loop is software-pipelined (stage B one channel behind stage
A) so PSUM evacuations hide behind a full channel of PE work.

Perf notes (from NTFF traces): engines only reach user code ~6.2us in
(double start barrier + per-engine TENSOR_LOAD), and data DMAs land
~9.6-10.5us in (DGE start latency), so a short burst of dummy matmuls
bridges the PE to data-ready; HAM un-throttling (1.2->2.4 GHz) fires
~5.5us after sustained PE activity begins either way, so bridging
further would idle-wait what cold real matmuls can half-speed through.
x is host-transposed to [h, c, w] so input DMA lines are sz*512B
contiguous. Input loads issue on the sync engine ALONE (sharing it
with output stores head-of-line-blocks input behind og waits); output
stores issue on ScalarE (hardware DGE; gpsimd's software DGE measured
~40% slower with a multi-us drain tail). Evacuation split fixed:
ScalarE takes the 1024-col rank-pair tile, VectorE the 512-col tile
plus the output add — per-op fixed costs (~250ns ScalarE, ~150ns
VectorE) make finer splits lose.
"""

import numpy as np

import concourse.bass as bass
import concourse.tile as tile
from concourse import mybir
from concourse.bass_utils import run_bass_kernel_spmd

N_CORES = 8
C, H, W = 64, 256, 256
KS, HALF = 11, 5
TRIM = 134              # streamed cols per k-tile (even width, 8B-aligned dst)
TRIM_OFF = (0, 122)     # dst col offset per k-tile; overlap accumulates in PSUM
GC = 8                  # channels per DMA group
REL_TARGET = 1.55e-2    # white-noise rel-err budget for eigen truncation
N_WARM = 13             # warm-up matmuls (256 cols each) bridging to data-ready


def _split_sync_waits(nc: bass.Bass, max_waits: int = 1):
    """walrus in this env encodes at most one sync-wait command per
    instruction; move excess waits onto preceding same-engine NOPs
    (engine queues are in-order, so semantics are preserved)."""
    for f in nc.m.functions:
        for bb in list(f.blocks):
            insts = list(bb.instructions)
            new_insts = []
            changed = False
            for inst in insts:
                si = inst.sync_info
                waits = list(si.on_wait) if si is not None and si.on_wait else []
                if len(waits) > max_waits:
                    extra, keep = waits[:-max_waits], waits[-max_waits:]
                    for w in extra:
                        nop = mybir.InstNoOp(
                            name=nc.get_next_instruction_name(), ins=[], outs=[]
                        )
                        nop.engine = inst.engine
                        nop.sync_info = mybir.SyncInfo(on_wait=[w], on_update=[])
                        nc.register_instruction(nop)
                        new_insts.append(nop)
                    si.on_wait = keep
                    changed = True
                new_insts.append(inst)
            if changed:
                bb.instructions = new_insts


def _host_filters(sigmas: np.ndarray, alpha: np.ndarray):
    """Eigendecompose the combined 2D smoothing operator.

    Returns (qa, qb, R, s): packed banded filter blocks for stage A / B,
    each (128, 2*R*TRIM) float16, plus the identity shift s.
    """
    al = alpha.astype(np.float64)
    wts = np.exp(al - al.max())
    wts /= wts.sum()

    gs = np.zeros((8, KS))
    gs[0, HALF] = 1.0                                   # identity (k=0)
    for i, k in enumerate((1, 2, 3, 5), start=1):       # avg pools
        gs[i, HALF - k : HALF + k + 1] = 1.0 / (2 * k + 1)
    ax = np.arange(KS, dtype=np.float64) - (KS - 1) / 2.0
    for i in range(3):                                  # gaussians
        s = abs(float(sigmas[i])) + 1e-6
        g = np.exp(-0.5 * (ax / s) ** 2)
        gs[5 + i] = g / g.sum()

    # The device graph unconditionally adds s*X (the host-prescaled input)
    # at output evacuation, so the eigen part must represent
    # M' = M - s*delta@delta. s is a free parameter: alternate eigh /
    # s = delta^T(M - rank_R)delta to minimize the rank-R residual, and
    # take the smallest R whose predicted white-noise rel err (residual
    # Frobenius over ||M||_F) fits the budget. Clamp s away from 0 so
    # qb = lam/s stays in fp16 range for degenerate softmax weights
    # (M' then goes indefinite, which the |lam| ordering handles).
    w0 = float(wts[0])
    M = (gs.T * wts) @ gs                               # 11x11, rank<=6
    MF = np.linalg.norm(M)
    delta = gs[0]
    for R in range(1, 7):
        s_id = min(max(w0, 1e-2), 1.0)
        for _ in range(60):
            Mr = M - s_id * np.outer(delta, delta)
            lam, V = np.linalg.eigh(Mr)
            order = np.argsort(-np.abs(lam))
            lam, V = lam[order], V[:, order]
            A = (V[:, :R] * lam[:R]) @ V[:, :R].T
            s_new = min(max(float((M - A)[HALF, HALF]), 1e-2), 1.0)
            if abs(s_new - s_id) < 1e-12:
                break
            s_id = s_new
        if np.sqrt(np.sum(lam[R:] ** 2)) < REL_TARGET * MF or R == 6:
            break

    def band(q):
        Q = np.zeros((H, H))
        for d in range(-HALF, HALF + 1):
            i = np.arange(max(0, -d), min(H, H - d))
            Q[i, i + d] = q[d + HALF]
        return Q

    def pack(mats):
        out = np.zeros((128, 2 * R * TRIM), np.float16)
        for kt in range(2):
            for r, Q in enumerate(mats):
                blk = Q[kt * 128 : (kt + 1) * 128, TRIM_OFF[kt] : TRIM_OFF[kt] + TRIM]
                out[:, (kt * R + r) * TRIM : (kt * R + r + 1) * TRIM] = blk.astype(
                    np.float16
                )
        return out

    qa = pack([band(V[:, r]) for r in range(R)])
    qb = pack([band(V[:, r] * (lam[r] / s_id)) for r in range(R)])
    return qa, qb, R, s_id


def _build_nc(R: int) -> bass.Bass:
    nc = bass.Bass()
    # x is host-transposed to [h, c, w] so each input DMA moves
    # sz-channel * 512B contiguous lines per partition; out stays
    # [c, h, w] (the evac add wants (c, t, w)-major SBUF layout).
    x = nc.declare_dram_parameter("x", [H, C, W], mybir.dt.float16, isOutput=False)
    qa = nc.declare_dram_parameter(
        "qa", [128, 2 * R * TRIM], mybir.dt.float16, isOutput=False
    )
    qb = nc.declare_dram_parameter(
        "qb", [128, 2 * R * TRIM], mybir.dt.float16, isOutput=False
    )
    out = nc.declare_dram_parameter("out", [C, H, W], mybir.dt.float16, isOutput=True)

    f16, f32 = mybir.dt.float16, mybir.dt.float32
    n_full = R // 2          # stage-A PSUM tiles holding 2 ranks (2 banks)
    has_half = R % 2         # plus one single-rank tile (1 bank)
    n_pairs = n_full + has_half
    # PSUM budget (8 banks): R=3 fits full double buffering
    # (2*2 + 2*1 + 2 = 8); R>=4 falls back to 1.5-channel buffering.
    exact_pools = 2 * (2 * n_full + has_half) + 2 <= 8

    with tile.TileContext(nc) as tc:
        with (
            tc.tile_pool(name="consts", bufs=1) as consts,
            tc.tile_pool(name="xin", bufs=3) as xin,
            tc.tile_pool(name="ysb", bufs=3) as ysb,
            tc.tile_pool(name="osb", bufs=2) as osb,
            tc.tile_pool(name="psaf", bufs=(2 if exact_pools else max(3, n_pairs)), space="PSUM") as psaf,
            tc.tile_pool(name="psah", bufs=2, space="PSUM") as psah,
            tc.tile_pool(name="pso", bufs=2, space="PSUM") as pso,
        ):
            qa_sb = consts.tile([128, 2 * R * TRIM], f16)
            qb_sb = consts.tile([128, 2 * R * TRIM], f16)
            nc.scalar.dma_start(out=qa_sb[:, :], in_=qa[:, :])
            nc.scalar.dma_start(out=qb_sb[:, :], in_=qb[:, :])

            # PE warm-up: engines reach user code ~6.2us in and data DMAs
            # land ~10us in; HAM un-throttles (1.2->2.4 GHz) ~5.5us after
            # sustained PE activity starts. Bridge the PE to data-ready
            # with dummy matmuls (PSUM never read) — no further, since
            # cold real matmuls still make half-speed progress.
            scratch = consts.tile([128, 256], f16, name="scratch")
            nc.gpsimd.memset(scratch[:, :], 0.0)
            wtile = [
                pso.tile([128, 512], f32, name=f"warm{i}", tag="po") for i in range(2)
            ]
            for i in range(N_WARM):
                nc.tensor.matmul(
                    wtile[i % 2][:, 0:256],
                    lhsT=scratch[:, 0:128],
                    rhs=scratch[:, 0:256],
                    start=True,
                    stop=True,
                )
            # input groups: small first so PE starts early; output groups:
            # small last so the final store DMA is short
            in_sizes = [1, 2, 2, 3] + [GC] * ((C - 8) // GC)
            out_sizes = [GC] * ((C - 8) // GC) + [4, 2, 1, 1]
            def group_map(sizes):
                m, start = {}, 0
                for gi, sz in enumerate(sizes):
                    for off in range(sz):
                        m[start + off] = (gi, off, start, sz)
                    start += sz
                return m
            in_map, out_map = group_map(in_sizes), group_map(out_sizes)

            xgs: dict[int, object] = {}
            ogs: dict[int, object] = {}
            ys_by_c: dict[int, list] = {}

            def stage_a(c):
                g, ci, c0, sz = in_map[c]
                if ci == 0:
                    # xg[p, (t, c, w)] = x[t*128+p, c, w]; one DMA per
                    # k-tile t keeps (c, w) lines contiguous on both sides.
                    xg = xin.tile([128, sz * 512], f16, name=f"xg{g}", tag="xg")
                    for t in range(2):
                        nc.sync.dma_start(
                            out=xg[:, t * sz * 256 : (t + 1) * sz * 256].rearrange(
                                "p (c w) -> p c w", c=sz
                            ),
                            in_=x[t * 128 : (t + 1) * 128, c0 : c0 + sz, :],
                        )
                    xgs[g] = xg
                xg = xgs[g]
                # stage A: Y_r^T = X^T Q_r (contract H on partitions). Rank
                # pair (2j, 2j+1) shares one 2-bank PSUM tile; (kt, mt)
                # outer so consecutive MMs share the stationary X chunk.
                pas = [
                    psaf.tile([128, 1024], f32, name=f"pa{j}", tag="pa")
                    for j in range(n_full)
                ]
                if has_half:
                    pas.append(
                        psah.tile([128, 512], f32, name=f"pa{n_full}", tag="pah")
                        if exact_pools
                        else psaf.tile([128, 1024], f32, name=f"pa{n_full}", tag="pa")
                    )
                for kt in range(2):
                    for mt in range(2):
                        base = kt * sz * 256 + ci * 256 + mt * 128
                        lhs = xg[:, base : base + 128]
                        for r in range(R):
                            dst = (r % 2) * 512 + mt * 256 + TRIM_OFF[kt]
                            nc.tensor.matmul(
                                pas[r // 2][:, dst : dst + TRIM],
                                lhsT=lhs,
                                rhs=qa_sb[
                                    :, (kt * R + r) * TRIM : (kt * R + r + 1) * TRIM
                                ],
                                start=(kt == 0 and mt == 0),
                                stop=(kt == 1 and mt == 1),
                            )
                # evacuate PSUM -> SBUF f16: ScalarE takes the rank-pair
                # tiles, VectorE the half tile (it also owns the output
                # adds); both average under the PE's per-channel time.
                ys = []
                for j in range(n_pairs):
                    width = 1024 if 2 * j + 1 < R else 512
                    y = ysb.tile([128, width], f16, name=f"y{j}", tag=f"y{width}")
                    if width == 1024 or n_pairs == 1:
                        nc.scalar.copy(out=y[:, :width], in_=pas[j][:, :width])
                    else:
                        nc.vector.tensor_copy(out=y[:, :width], in_=pas[j][:, :width])
                    ys.append(y)
                ys_by_c[c] = ys

            def stage_b(c):
                g, ci, c0, sz = out_map[c]
                if ci == 0:
                    ogs[g] = osb.tile([128, sz * 512], f16, name=f"og{g}", tag="og")
                ys = ys_by_c.pop(c)
                # stage B: out = sum_r Y_r (lam_r Q_r)  (contract W)
                po = pso.tile([128, 512], f32, tag="po")
                for r in range(R):
                    for kt in range(2):
                        for mt in range(2):
                            dst = mt * 256 + TRIM_OFF[kt]
                            src = (r % 2) * 512 + kt * 256 + mt * 128
                            nc.tensor.matmul(
                                po[:, dst : dst + TRIM],
                                lhsT=ys[r // 2][:, src : src + 128],
                                rhs=qb_sb[
                                    :, (kt * R + r) * TRIM : (kt * R + r + 1) * TRIM
                                ],
                                start=(r == 0 and kt == 0 and mt == 0),
                                stop=(r == R - 1 and kt == 1 and mt == 1),
                            )
                gi, cii, _, szi = in_map[c]
                # og[p, (c, t, w)] = po + s*X in ONE VectorE op: po cols
                # are (t, w) matching og's within-channel layout; xg is
                # t-major, so its operand is a strided 2-run 3D AP.
                nc.vector.tensor_add(
                    ogs[g][:, ci * 512 : (ci + 1) * 512].rearrange(
                        "p (t w) -> p t w", t=2
                    ),
                    po[:, 0:512].rearrange("p (t w) -> p t w", t=2),
                    xgs[gi][:, :].rearrange("p (t c w) -> p t c w", t=2, c=szi)[
                        :, :, cii, :
                    ],
                )
                if ci == sz - 1:
                    nc.scalar.dma_start(
                        out=out[c0 : c0 + sz].rearrange("c (t p) w -> p c t w", p=128),
                        in_=ogs.pop(g)[:, :].rearrange(
                            "p (c t w) -> p c t w", c=sz, t=2
                        ),
                    )

            # software pipeline: B(c-1) is emitted after A(c), so stage-A
            # evacuations have a full channel of PE work to hide behind
            for c in range(C):
                stage_a(c)
                if c > 0:
                    stage_b(c - 1)
            stage_b(C - 1)
    _split_sync_waits(nc)
    return nc


_NC_CACHE: dict[int, bass.Bass] = {}


def _get_nc(R: int) -> bass.Bass:
    if R not in _NC_CACHE:
        _NC_CACHE[R] = _build_nc(R)
    return _NC_CACHE[R]


def _run(x, sigmas, alpha, trace=False):
    qa, qb, R, s_id = _host_filters(np.asarray(sigmas), np.asarray(alpha))
    # device computes (1/s) * (sum_r Q_r (s X) (lam_r/s) Q_r + s X);
    # scaling X by s up front makes the identity term a plain add at evac
    x = (np.asarray(x, dtype=np.float32) * np.float32(s_id)).astype(np.float16)
    # [b, c, h, w] -> per-core [h, c, w] so input DMA lines are contiguous
    xt = np.ascontiguousarray(x.transpose(0, 2, 1, 3))
    nc = _get_nc(R)
    in_maps = [{"x": xt[i], "qa": qa, "qb": qb} for i in range(N_CORES)]
    res = run_bass_kernel_spmd(
        nc, in_maps, core_ids=list(range(N_CORES)), trace=trace
    )
    out = np.stack([res.results[i]["out"] for i in range(N_CORES)])
    return out.astype(np.float32), res.exec_time_ns


def kernel(x, sigmas, alpha):
    out, _ = _run(x, sigmas, alpha, trace=False)
    return out


# revision 10
# speedup vs baseline: 1.0681x; 1.0658x over previous
"""Trainium2 kernel for nn_AdaptivePoolOrGaussian.

Reference computes, per (batch, channel) image X (256x256):
    out = sum_i w_i * (K_i conv X),  w = softmax(alpha)
where the 8 K_i are separable symmetric 11-tap 2D kernels
(5 avg-pools incl. identity + 3 Gaussians), zero-padded "same" convs.

Math: all 8 tap vectors are even-symmetric 11-vectors, which span a
6-dim space, so the combined operator M = sum_i w_i g_i g_i^T (11x11,
PSD) has rank <= 6. The identity (k=0 pool) term is peeled off and
applied exactly as "+ w0*X" during output evacuation; the smooth
remainder is eigendecomposed on the host, M_rest ~= sum_r lam_r q_r
q_r^T (R=4 keeps rel err ~2.5e-3), giving
    out = sum_r lam_r * conv_H(q_r) conv_W(q_r) X + w0 X.
Each 1D conv along a 256-long axis is a banded 256x256 matmul; band
structure lets each 128-row k-tile stream only 134 of 256 output
columns. Sharding is pure data parallel: core i owns batch element i.
Per channel: stage A (conv H) matmuls X^T Q_r into PSUM, evacuate to
SBUF fp16; stage B (conv W) accumulates sum_r Y_r (lam_r/w0 Q_r) in
PSUM over all ranks, then VectorE adds the prescaled w0*X during the
PSUM->SBUF copy. Compute dtype fp16, PSUM accumulates fp32. The
channel loop is software-pipelined (stage B one channel behind stage
A) so PSUM evacuations hide behind a full channel of PE work.
"""

import numpy as np

import concourse.bass as bass
import concourse.tile as tile
from concourse import mybir
from concourse.bass_utils import run_bass_kernel_spmd

N_CORES = 8
C, H, W = 64, 256, 256
KS, HALF = 11, 5
TRIM = 134              # streamed cols per k-tile (even width, 8B-aligned dst)
TRIM_OFF = (0, 122)     # dst col offset per k-tile; overlap accumulates in PSUM
GC = 8                  # channels per DMA group
REL_TARGET = 1.55e-2    # white-noise rel-err budget for eigen truncation


def _split_sync_waits(nc: bass.Bass, max_waits: int = 1):
    """walrus in this env encodes at most one sync-wait command per
    instruction; move excess waits onto preceding same-engine NOPs
    (engine queues are in-order, so semantics are preserved)."""
    for f in nc.m.functions:
        for bb in list(f.blocks):
            insts = list(bb.instructions)
            new_insts = []
            changed = False
            for inst in insts:
                si = inst.sync_info
                waits = list(si.on_wait) if si is not None and si.on_wait else []
                if len(waits) > max_waits:
                    extra, keep = waits[:-max_waits], waits[-max_waits:]
                    for w in extra:
                        nop = mybir.InstNoOp(
                            name=nc.get_next_instruction_name(), ins=[], outs=[]
                        )
                        nop.engine = inst.engine
                        nop.sync_info = mybir.SyncInfo(on_wait=[w], on_update=[])
                        nc.register_instruction(nop)
                        new_insts.append(nop)
                    si.on_wait = keep
                    changed = True
                new_insts.append(inst)
            if changed:
                bb.instructions = new_insts


def _host_filters(sigmas: np.ndarray, alpha: np.ndarray):
    """Eigendecompose the combined 2D smoothing operator.

    Returns (qa, qb, R): packed banded filter blocks for stage A / B,
    each (128, 2*R*TRIM) float16.
    """
    al = alpha.astype(np.float64)
    wts = np.exp(al - al.max())
    wts /= wts.sum()

    gs = np.zeros((8, KS))
    gs[0, HALF] = 1.0                                   # identity (k=0)
    for i, k in enumerate((1, 2, 3, 5), start=1):       # avg pools
        gs[i, HALF - k : HALF + k + 1] = 1.0 / (2 * k + 1)
    ax = np.arange(KS, dtype=np.float64) - (KS - 1) / 2.0
    for i in range(3):                                  # gaussians
        s = abs(float(sigmas[i])) + 1e-6
        g = np.exp(-0.5 * (ax / s) ** 2)
        gs[5 + i] = g / g.sum()

    # The device graph unconditionally adds s*X (the host-prescaled input)
    # at output evacuation, so the eigen part must represent
    # M' = M - s*delta@delta. s is a free parameter: alternate eigh /
    # s = delta^T(M - rank_R)delta to minimize the rank-R residual, and
    # take the smallest R whose predicted white-noise rel err (residual
    # Frobenius over ||M||_F) fits the budget. Clamp s away from 0 so
    # qb = lam/s stays in fp16 range for degenerate softmax weights
    # (M' then goes indefinite, which the |lam| ordering handles).
    w0 = float(wts[0])
    M = (gs.T * wts) @ gs                               # 11x11, rank<=6
    MF = np.linalg.norm(M)
    delta = gs[0]
    for R in range(1, 7):
        s_id = min(max(w0, 1e-2), 1.0)
        for _ in range(60):
            Mr = M - s_id * np.outer(delta, delta)
            lam, V = np.linalg.eigh(Mr)
            order = np.argsort(-np.abs(lam))
            lam, V = lam[order], V[:, order]
            A = (V[:, :R] * lam[:R]) @ V[:, :R].T
            s_new = min(max(float((M - A)[HALF, HALF]), 1e-2), 1.0)
            if abs(s_new - s_id) < 1e-12:
                break
            s_id = s_new
        if np.sqrt(np.sum(lam[R:] ** 2)) < REL_TARGET * MF or R == 6:
            break
    w0 = s_id

    def band(q):
        Q = np.zeros((H, H))
        for d in range(-HALF, HALF + 1):
            i = np.arange(max(0, -d), min(H, H - d))
            Q[i, i + d] = q[d + HALF]
        return Q

    def pack(mats):
        out = np.zeros((128, 2 * R * TRIM), np.float16)
        for kt in range(2):
            for r, Q in enumerate(mats):
                blk = Q[kt * 128 : (kt + 1) * 128, TRIM_OFF[kt] : TRIM_OFF[kt] + TRIM]
                out[:, (kt * R + r) * TRIM : (kt * R + r + 1) * TRIM] = blk.astype(
                    np.float16
                )
        return out

    qa = pack([band(V[:, r]) for r in range(R)])
    qb = pack([band(V[:, r] * (lam[r] / w0)) for r in range(R)])
    return qa, qb, R, w0


def _build_nc(R: int) -> bass.Bass:
    nc = bass.Bass()
    x = nc.declare_dram_parameter("x", [C, H, W], mybir.dt.float16, isOutput=False)
    qa = nc.declare_dram_parameter(
        "qa", [128, 2 * R * TRIM], mybir.dt.float16, isOutput=False
    )
    qb = nc.declare_dram_parameter(
        "qb", [128, 2 * R * TRIM], mybir.dt.float16, isOutput=False
    )
    out = nc.declare_dram_parameter("out", [C, H, W], mybir.dt.float16, isOutput=True)

    f16, f32 = mybir.dt.float16, mybir.dt.float32
    n_pairs = (R + 1) // 2  # stage-A PSUM tiles hold 2 ranks (2 banks) each

    with tile.TileContext(nc) as tc:
        with (
            tc.tile_pool(name="consts", bufs=1) as consts,
            tc.tile_pool(name="xin", bufs=3) as xin,
            tc.tile_pool(name="ysb", bufs=2 * n_pairs + 1) as ysb,
            tc.tile_pool(name="osb", bufs=2) as osb,
            tc.tile_pool(name="psa", bufs=max(3, n_pairs), space="PSUM") as psa,
            tc.tile_pool(name="pso", bufs=2, space="PSUM") as pso,
        ):
            qa_sb = consts.tile([128, 2 * R * TRIM], f16)
            qb_sb = consts.tile([128, 2 * R * TRIM], f16)
            nc.gpsimd.dma_start(out=qa_sb[:, :], in_=qa[:, :])
            nc.gpsimd.dma_start(out=qb_sb[:, :], in_=qb[:, :])

            # PE clock warm-up: HAM starts the PE at 1.2 GHz and only
            # un-throttles after ~3.4us of sustained activity. Burn dummy
            # matmuls on a zeroed scratch tile while the head DMAs land so
            # the real matmuls start at 2.4 GHz.
            scratch = consts.tile([128, 128], f16, name="scratch")
            nc.gpsimd.memset(scratch[:, :], 0.0)
            warm = psa.tile([128, 1024], f32, name="warm", tag="pa")
            for i in range(26):
                nc.tensor.matmul(
                    warm[:, 0:128],
                    lhsT=scratch[:, 0:128],
                    rhs=scratch[:, 0:128],
                    start=(i == 0),
                    stop=(i == 25),
                )
            eng = [
                nc.scalar.copy,
                lambda out, in_: nc.vector.tensor_copy(out, in_),
            ]
            # input groups: small first so PE starts early; output groups:
            # small last so the final store DMA is short
            in_sizes = [1, 2, 2, 3] + [GC] * ((C - 8) // GC)
            out_sizes = [GC] * ((C - 8) // GC) + [4, 2, 1, 1]
            def group_map(sizes):
                m, start = {}, 0
                for gi, sz in enumerate(sizes):
                    for off in range(sz):
                        m[start + off] = (gi, off, start, sz)
                    start += sz
                return m
            in_map, out_map = group_map(in_sizes), group_map(out_sizes)

            xgs: dict[int, object] = {}
            ogs: dict[int, object] = {}
            ys_by_c: dict[int, list] = {}

            def stage_a(c):
                g, ci, c0, sz = in_map[c]
                if ci == 0:
                    # x[c, kt*128+p, w] -> xg[p, (dc, kt, w)]
                    xg = xin.tile([128, sz * 512], f16, name=f"xg{g}", tag="xg")
                    nc.sync.dma_start(
                        out=xg[:, :].rearrange("p (c t w) -> p c t w", c=sz, t=2),
                        in_=x[c0 : c0 + sz].rearrange("c (t p) w -> p c t w", p=128),
                    )
                    xgs[g] = xg
                xg = xgs[g]
                # stage A: Y_r^T = X^T Q_r (contract H on partitions). Rank
                # pair (2j, 2j+1) shares one 2-bank PSUM tile; (kt, mt)
                # outer so consecutive MMs share the stationary X chunk.
                pas = [
                    psa.tile([128, 1024], f32, name=f"pa{j}", tag="pa")
                    for j in range(n_pairs)
                ]
                for kt in range(2):
                    for mt in range(2):
                        base = ci * 512 + kt * 256 + mt * 128
                        lhs = xg[:, base : base + 128]
                        for r in range(R):
                            dst = (r % 2) * 512 + mt * 256 + TRIM_OFF[kt]
                            nc.tensor.matmul(
                                pas[r // 2][:, dst : dst + TRIM],
                                lhsT=lhs,
                                rhs=qa_sb[
                                    :, (kt * R + r) * TRIM : (kt * R + r + 1) * TRIM
                                ],
                                start=(kt == 0 and mt == 0),
                                stop=(kt == 1 and mt == 1),
                            )
                # evacuate PSUM -> SBUF f16, alternating ScalarE/VectorE
                ys = [
                    ysb.tile([128, 1024], f16, name=f"y{j}", tag="y")
                    for j in range(n_pairs)
                ]
                for j in range(n_pairs):
                    width = 1024 if 2 * j + 1 < R else 512
                    e = 0 if j == 0 else (0 if c % 3 == 0 else 1)
                    eng[e](out=ys[j][:, :width], in_=pas[j][:, :width])
                ys_by_c[c] = ys

            def stage_b(c):
                g, ci, c0, sz = out_map[c]
                if ci == 0:
                    ogs[g] = osb.tile([128, sz * 512], f16, name=f"og{g}", tag="og")
                ys = ys_by_c.pop(c)
                # stage B: out = sum_r Y_r (lam_r Q_r)  (contract W)
                po = pso.tile([128, 512], f32)
                for r in range(R):
                    for kt in range(2):
                        for mt in range(2):
                            dst = mt * 256 + TRIM_OFF[kt]
                            src = (r % 2) * 512 + kt * 256 + mt * 128
                            nc.tensor.matmul(
                                po[:, dst : dst + TRIM],
                                lhsT=ys[r // 2][:, src : src + 128],
                                rhs=qb_sb[
                                    :, (kt * R + r) * TRIM : (kt * R + r + 1) * TRIM
                                ],
                                start=(r == 0 and kt == 0 and mt == 0),
                                stop=(r == R - 1 and kt == 1 and mt == 1),
                            )
                gi, cii = in_map[c][0], in_map[c][1]
                nc.vector.tensor_add(
                    ogs[g][:, ci * 512 : (ci + 1) * 512],
                    po[:, :],
                    xgs[gi][:, cii * 512 : (cii + 1) * 512],
                )
                if ci == sz - 1:
                    nc.sync.dma_start(
                        out=out[c0 : c0 + sz].rearrange("c (t p) w -> p c t w", p=128),
                        in_=ogs.pop(g)[:, :].rearrange(
                            "p (c t w) -> p c t w", c=sz, t=2
                        ),
                    )

            # software pipeline: B(c-1) is emitted after A(c), so stage-A
            # evacuations have a full channel of PE work to hide behind
            for c in range(C):
                stage_a(c)
                if c > 0:
                    stage_b(c - 1)
            stage_b(C - 1)
    _split_sync_waits(nc)
    return nc


_NC_CACHE: dict[int, bass.Bass] = {}


def _get_nc(R: int) -> bass.Bass:
    if R not in _NC_CACHE:
        _NC_CACHE[R] = _build_nc(R)
    return _NC_CACHE[R]


def _run(x, sigmas, alpha, trace=False):
    qa, qb, R, w0 = _host_filters(np.asarray(sigmas), np.asarray(alpha))
    # device computes (1/w0) * (sum_r Q_r (w0 X) (lam_r/w0) Q_r + w0 X);
    # scaling X by w0 up front makes the identity term a plain add at evac
    x = (np.asarray(x, dtype=np.float32) * np.float32(w0)).astype(np.float16)
    x = np.ascontiguousarray(x)
    nc = _get_nc(R)
    in_maps = [
        {"x": np.ascontiguousarray(x[i]), "qa": qa, "qb": qb} for i in range(N_CORES)
    ]
    res = run_bass_kernel_spmd(
        nc, in_maps, core_ids=list(range(N_CORES)), trace=trace
    )
    out = np.stack([res.results[i]["out"] for i in range(N_CORES)])
    return out.astype(np.float32), res.exec_time_ns


def kernel(x, sigmas, alpha):
    out, _ = _run(x, sigmas, alpha, trace=False)
    return out

